# revision 45
# baseline (speedup 1.0000x reference)
"""Trainium2 Bass kernel: LayerNorm -> MHA(16 heads, S=4096, D=1024) -> out-proj.

Sharding: tensor-parallel over heads. 8 cores x 2 heads each.
Each core computes LN(x) (replicated), q/k/v for its 2 heads (columns of
Wq/Wk/Wv), attention for those heads, and a partial output projection
(its 128 rows of Wo.T) in bf16. Host sums the 8 partials and adds bo.

v3 notes (what profiling taught us):
  - The chip power-manages the PE: sustained-dense phases run at an
    effective ~1.2 GHz (hardware duty cap k=4/8 in the NTFF 'ham' spans),
    light phases at 2.4 GHz. Dense attention streaming is therefore
    ~427ns per 512-col pass no matter the dtype (fp32r/bf16/fp8 all
    measured the same) -- so everything is 16-bit for margin, not speed.
  - The t-loop is kept minimal (8 matmuls + 2 exps per t); evacuation,
    softmax-sum reciprocal, out-projection and output DMA all live
    outside the dense loop where the clock is granted in full.
  - Softmax normalization is applied to out-projection ROWS (per-head
    1/sum scaling on PSUM evacuation: ACT does po0*r0, DVE does
    po1*r1 + that), which deletes v1's per-chunk transpose-normalize.
  - LN apply alternates DVE/ACT per tile to balance engine load.

Per-core layout (core c, heads 2c, 2c+1; d-slice = [128c, 128c+128)):
  phase 1: LN in [m,d] tiles -> PE-transpose -> hT [d,m] bf16;
           q/k = W.T @ hT + b -> bf16 [128d, S]; v -> transpose ->
           [t, (tc,head,65)] fp16 (ones col accumulates softmax sums)
  phase 2: scoresT[t,m] = kT.T @ qT per head (K=64, disjoint PE row
           tiles) -> exp (ACT, scale=1/32) -> fp16 w;
           ctx_u[65,m] += v_aug.T @ w accumulated over t in PSUM;
           per-mac: ctx_u -> sbuf bf16, sums -> transpose -> 1/sums.
  phase 3: po_h[m,e] = ctx_u_h.T @ woT_h; out = po0*r0 + po1*r1 (bf16)
           -> DMA per 128-row chunk.

LN gain g is folded into Wq/Wk/Wv columns host-side; LN bias b_ln is folded
into bq/bk/bv.  bo is added host-side after the cross-core reduction.
"""

import math
import os
from contextlib import ExitStack

import numpy as np

B, S, DIM, H = 1, 4096, 1024, 16
HD = DIM // H            # 64
N_CORES = 8
HPC = H // N_CORES       # 2 heads per core
DC = HPC * HD            # 128 dims per core
MB = 512                 # phase-1 m-block
N_MB = S // MB           # 8
MAC = 1024               # phase-2 m-macro
N_MAC = S // MAC         # 4
TC = S // 128            # 32 t-chunks
SCALE = 1.0 / math.sqrt(DIM)

FP8 = os.environ.get("FP8", "0") == "1"   # fp8 q/k: no speed gain measured,
                                          # costs error margin; off by default
_CACHE = {}
LAST_RESULT = None       # BassKernelResults of the most recent run (for test.py)


def _build():
    import concourse.bacc as bacc
    import concourse.tile as tile
    import concourse.mybir as mybir
    from concourse.masks import make_identity

    dt = mybir.dt
    AF = mybir.ActivationFunctionType
    ALU = mybir.AluOpType

    qk_dt = dt.float8e4 if FP8 else dt.bfloat16

    nc = bacc.Bacc("TRN2", target_bir_lowering=False, debug=False,
                   num_devices=N_CORES)

    x_d = nc.dram_tensor("x", [S, DIM], dt.float32, kind="ExternalInput")
    wqT_d = nc.dram_tensor("wqT", [DIM, DC], dt.bfloat16, kind="ExternalInput")
    wkT_d = nc.dram_tensor("wkT", [DIM, DC], dt.bfloat16, kind="ExternalInput")
    wvT_d = nc.dram_tensor("wvT", [DIM, DC], dt.bfloat16, kind="ExternalInput")
    woT_d = nc.dram_tensor("woT", [DC, DIM], dt.bfloat16, kind="ExternalInput")
    bq_d = nc.dram_tensor("bq", [DC], dt.float32, kind="ExternalInput")
    bk_d = nc.dram_tensor("bk", [DC], dt.float32, kind="ExternalInput")
    bv_d = nc.dram_tensor("bv", [DC], dt.float32, kind="ExternalInput")
    out_d = nc.dram_tensor("out", [S, DIM], dt.bfloat16, kind="ExternalOutput")

    with tile.TileContext(nc) as tc, ExitStack() as top:
        persist = top.enter_context(tc.tile_pool(name="persist", bufs=1))

        ident = persist.tile([128, 128], dt.float32)
        ident_bf = persist.tile([128, 128], dt.bfloat16)
        eps_t = persist.tile([128, 1], dt.float32)
        ones_bf = persist.tile([128, HD], dt.bfloat16)

        wT = {n: persist.tile([128, DIM // 128, DC], dt.bfloat16,
                              tag=f"w{n}T", name=f"w{n}T")
              for n in ("q", "k", "v")}
        woT = {h: persist.tile([HD, DIM], dt.bfloat16, tag=f"woT{h}",
                               name=f"woT{h}")
               for h in range(HPC)}
        bias = {n: persist.tile([DC, 1], dt.float32, tag=f"b{n}", name=f"b{n}")
                for n in ("q", "k", "v")}

        qf = persist.tile([DC, S], qk_dt, tag="qf")
        kf = persist.tile([DC, S], qk_dt, tag="kf")
        # v with an appended ones-column per head: [t-part, tc, head, HD+1]
        v_all = persist.tile([128, TC, HPC, HD + 1], dt.float16)

        # ---------------- phase 1: LN + QKV projections ----------------
        with ExitStack() as p1:
            xpool = p1.enter_context(tc.tile_pool(name="xp", bufs=10))
            hpool = p1.enter_context(tc.tile_pool(name="hp", bufs=5))
            hTpool = p1.enter_context(tc.tile_pool(name="hTp", bufs=3))
            stat = p1.enter_context(tc.tile_pool(name="stat", bufs=8))
            vsb = p1.enter_context(tc.tile_pool(name="vsb", bufs=2))
            ps_t = p1.enter_context(tc.tile_pool(name="ps_t", bufs=2, space="PSUM"))
            ps_p = p1.enter_context(tc.tile_pool(name="ps_p", bufs=4, space="PSUM"))
            ps_v = p1.enter_context(tc.tile_pool(name="ps_v", bufs=2, space="PSUM"))

            # first two m-blocks' x tiles: DMA queued before anything else,
            # split in column quarters so each tile arrives via 4 queues
            def load_x(r0, name=None, way=2):
                xt = xpool.tile([128, DIM], dt.float32, tag="x",
                                name=name or "xt")
                cw = DIM // way
                for c in range(way):
                    cs = slice(c * cw, (c + 1) * cw)
                    nc.sync.dma_start(out=xt[:, cs],
                                      in_=x_d.ap()[r0:r0 + 128, cs])
                return xt

            first_x = {}
            for mb in range(2):
                for j in range(MB // 128):
                    first_x[(mb, j)] = load_x(mb * MB + j * 128, name="xt0")

            make_identity(nc, ident)
            nc.vector.tensor_copy(out=ident_bf, in_=ident)
            nc.vector.memset(eps_t, 1e-5)
            nc.vector.memset(ones_bf, 1.0)
            nc.vector.memset(v_all, 1.0)
            for n, d in (("q", wqT_d), ("k", wkT_d), ("v", wvT_d)):
                nc.sync.dma_start(out=wT[n], in_=d.ap().rearrange(
                    "(c p) n -> p c n", p=128))
            for h in range(HPC):
                nc.sync.dma_start(out=woT[h],
                                  in_=woT_d.ap()[h * HD:(h + 1) * HD, :])
            for n, d in (("q", bq_d), ("k", bk_d), ("v", bv_d)):
                nc.sync.dma_start(out=bias[n], in_=d.ap()[:, None])

            for mb in range(N_MB):
                hs = []
                for j in range(MB // 128):
                    if (mb, j) in first_x:
                        xt = first_x[(mb, j)]
                    else:
                        xt = load_x(mb * MB + j * 128)
                    st = stat.tile([128, 2, nc.vector.BN_STATS_DIM],
                                   dt.float32, tag="st")
                    xg = xt[:].rearrange("p (s f) -> p s f", s=2)
                    for sg in range(2):
                        nc.vector.bn_stats(out=st[:, sg, :], in_=xg[:, sg, :])
                    mv = stat.tile([128, 2], dt.float32, tag="mv")
                    nc.vector.bn_aggr(out=mv, in_=st)
                    std = stat.tile([128, 1], dt.float32, tag="sd")
                    nc.scalar.activation(out=std, in_=mv[:, 1:2], func=AF.Sqrt,
                                         bias=eps_t, scale=1.0)
                    rstd = stat.tile([128, 1], dt.float32, tag="rs")
                    nc.vector.reciprocal(out=rstd, in_=std)
                    ht = hpool.tile([128, DIM], dt.bfloat16, tag="h")
                    if j % 2 == 0:
                        # ACT path: h = Copy(rstd*x + (-mu*rstd))
                        nb = stat.tile([128, 1], dt.float32, tag="nb")
                        nc.vector.tensor_scalar(
                            out=nb, in0=mv[:, 0:1], scalar1=rstd,
                            scalar2=-1.0, op0=ALU.mult, op1=ALU.mult)
                        nc.scalar.activation(out=ht, in_=xt, func=AF.Identity,
                                             bias=nb, scale=rstd)
                    else:
                        nc.vector.tensor_scalar(
                            out=ht, in0=xt, scalar1=mv[:, 0:1],
                            scalar2=rstd, op0=ALU.subtract, op1=ALU.mult)
                    hs.append(ht)

                # transpose h -> hT  [128d, dc, 512m]  (bf16)
                hT = hTpool.tile([128, DIM // 128, MB], dt.bfloat16, tag="hT")
                for dc in range(DIM // 128):
                    pt = ps_t.tile([128, MB], dt.bfloat16, tag="pt")
                    for j in range(MB // 128):
                        nc.tensor.transpose(
                            pt[:, j * 128:(j + 1) * 128],
                            hs[j][:, dc * 128:(dc + 1) * 128], ident_bf)
                    nc.scalar.copy(out=hT[:, dc, :], in_=pt)

                mbs = slice(mb * MB, (mb + 1) * MB)
                for name in ("q", "k", "v"):
                    pp = ps_p.tile([128, MB], dt.float32, tag="pp")
                    for dc in range(DIM // 128):
                        nc.tensor.matmul(pp, lhsT=wT[name][:, dc, :],
                                         rhs=hT[:, dc, :],
                                         start=(dc == 0), stop=(dc == 7))
                    if name != "v":
                        dest = qf if name == "q" else kf
                        nc.vector.tensor_scalar(
                            out=dest[:, mbs], in0=pp,
                            scalar1=bias[name], scalar2=None, op0=ALU.add)
                    else:
                        vT = vsb.tile([128, MB], dt.bfloat16, tag="vT")
                        nc.vector.tensor_scalar(
                            out=vT, in0=pp, scalar1=bias[name], scalar2=None,
                            op0=ALU.add)
                        pv = ps_v.tile([128, MB], dt.bfloat16, tag="pv")
                        for j in range(MB // 128):
                            nc.tensor.transpose(
                                pv[:, j * 128:(j + 1) * 128],
                                vT[:, j * 128:(j + 1) * 128], ident_bf)
                        for j in range(MB // 128):
                            tc_j = mb * (MB // 128) + j
                            src = pv[:, j * 128:(j + 1) * 128].rearrange(
                                "p (h e) -> p h e", h=HPC)
                            nc.vector.tensor_copy(
                                out=v_all[:, tc_j, :, 0:HD], in_=src)

        # ---------------- phase 2: attention (dense t-loop) ----------------
        # scores/exp use SEPARATE 512-col psum tiles per half (tags sa/sb):
        # whole-tile dependency tracking then lets exp(j) start right after
        # its own scores half, and AV(j) right after its own exp half.
        with ExitStack() as p2:
            spool = {j: p2.enter_context(
                tc.tile_pool(name=f"sp{j}", bufs=2, space="PSUM"))
                for j in range(MAC // 512)}
            cpool = p2.enter_context(tc.tile_pool(name="cp", bufs=2, space="PSUM"))
            wpool = {j: p2.enter_context(tc.tile_pool(name=f"wp{j}", bufs=4))
                     for j in range(MAC // 512)}
            upool = p2.enter_context(tc.tile_pool(name="up", bufs=3))
            rpool = p2.enter_context(tc.tile_pool(name="rp", bufs=2))
            opool = p2.enter_context(tc.tile_pool(name="op", bufs=3))
            dpool = p2.enter_context(tc.tile_pool(name="dp", bufs=4))

            # exp(s/32) = (c0 + c1p*s + c2p*s^2)^2 -- DVE offload for a
            # fraction of the exp tiles (ACT is the dense-loop bottleneck).
            # Quadratic fit of exp on |s|/64 <= 0.31; max rel err < 0.5%
            # at 6 sigma of the score distribution, ~0.2% typical.
            C0, C1P, C2P = 0.99996689620413, 0.01577568008520995, 1.2291052017642632e-4
            DVE_EXP_EVERY = 16

            cu_t = {}
            cun_t = {}
            pcu_t = {}
            ot_t = {}

            def evac_slot(mac, slot):
                """Deferred mac-boundary work, one light piece per t-slot of
                the following mac: 0 -> psum->sbuf ctx_u copies; 1/2 -> per
                head: r = 1/sums row, rbc = ones^T r (PE broadcast down 64
                partitions), ctx_n = ctx_u * rbc. Normalizing ctx along the
                free (m) dim here removes both the per-chunk sums transposes
                and any per-partition scaling in the out-projection."""
                if slot == 0:
                    for h in range(HPC):
                        cu = upool.tile([HD + 1, MAC], dt.bfloat16, tag="cu",
                                        name=f"cu{h}")
                        nc.vector.tensor_copy(out=cu, in_=pcu_t[(mac, h)])
                        cu_t[(mac, h)] = cu
                elif slot in (1, 2):
                    h = slot - 1
                    cu = cu_t[(mac, h)]
                    rr = rpool.tile([128, MAC], dt.bfloat16, tag="rr",
                                    name="rr")
                    with nc.allow_low_precision(
                            reason="softmax sums ~4e3; bf16 recip = 0.4%"):
                        nc.vector.reciprocal(out=rr[HD:HD + 1, :],
                                             in_=cu[HD:HD + 1, :])
                    rbc = cpool.tile([HD, MAC], dt.float32, tag="pc",
                                     name="rbc")
                    for j in range(MAC // 512):
                        js = slice(j * 512, (j + 1) * 512)
                        nc.tensor.matmul(rbc[:, js],
                                         lhsT=ones_bf[HD:HD + 1, :],
                                         rhs=rr[HD:HD + 1, js],
                                         start=True, stop=True)
                    cun = upool.tile([HD, MAC], dt.bfloat16, tag="cun",
                                     name=f"cun{h}", bufs=4)
                    nc.vector.tensor_tensor(out=cun, in0=cu[0:HD, :],
                                            in1=rbc, op=ALU.mult)
                    cun_t[(mac, h)] = cun

            for mac in range(N_MAC):
                m0 = mac * MAC
                for h in range(HPC):
                    pcu_t[(mac, h)] = cpool.tile([HD + 1, MAC], dt.float32,
                                                 tag="pc", name=f"pcu{h}")
                for t in range(TC):
                    ws = {}
                    for h in range(HPC):
                        hd0 = h * HD
                        for j in range(MAC // 512):
                            ps = spool[j].tile([128, 512], dt.float32,
                                               tag="s", name=f"ps{h}{j}")
                            nc.tensor.matmul(
                                ps,
                                lhsT=kf[hd0:hd0 + HD, t * 128:(t + 1) * 128],
                                rhs=qf[hd0:hd0 + HD,
                                       m0 + j * 512:m0 + (j + 1) * 512],
                                start=True, stop=True,
                                tile_position=(hd0, 0))
                            w = wpool[j].tile([128, 512], dt.float16,
                                              tag="w", name=f"w{h}{j}")
                            gidx = ((mac * TC + t) * HPC + h) * 2 + j
                            if gidx % DVE_EXP_EVERY == 8:
                                # u = c2p*s + c1p; v = s*u; vc = v + c0;
                                # w = vc^2  (each op reads PSUM at most once)
                                u = dpool.tile([128, 512], dt.float32,
                                               tag="dq", name="u")
                                nc.vector.tensor_scalar(
                                    out=u, in0=ps, scalar1=C2P, scalar2=C1P,
                                    op0=ALU.mult, op1=ALU.add)
                                v = dpool.tile([128, 512], dt.float32,
                                               tag="de", name="v")
                                nc.vector.scalar_tensor_tensor(
                                    out=v, in0=ps, scalar=0.0, in1=u,
                                    op0=ALU.add, op1=ALU.mult)
                                vc = dpool.tile([128, 512], dt.float32,
                                                tag="df", name="vc")
                                nc.vector.tensor_scalar(
                                    out=vc, in0=v, scalar1=C0, scalar2=None,
                                    op0=ALU.add)
                                nc.vector.tensor_tensor(out=w, in0=vc,
                                                        in1=vc, op=ALU.mult)
                            else:
                                nc.scalar.activation(out=w, in_=ps,
                                                     func=AF.Exp, scale=SCALE)
                            ws[(h, j)] = w
                    for h in range(HPC):
                        for j in range(MAC // 512):
                            js = slice(j * 512, (j + 1) * 512)
                            nc.tensor.matmul(
                                pcu_t[(mac, h)][:, js],
                                lhsT=v_all[:, t, h, :],
                                rhs=ws[(h, j)],
                                start=(t == 0), stop=(t == TC - 1),
                                skip_group_check=True)
                    if mac > 0:
                        evac_slot(mac - 1, t)

            # ---------------- phase 3: out-projection tail ----------------
            # ctx is pre-normalized, so both heads accumulate into one PSUM
            # tile and the evacuation is a plain copy (alternating DVE/ACT).
            for slot in range(3):
                evac_slot(N_MAC - 1, slot)
            for mac in range(N_MAC):
                for mc in range(MAC // 128):
                    ot = opool.tile([128, DIM], dt.bfloat16, tag="ot",
                                    name="ot")
                    for e in range(DIM // 512):
                        es = slice(e * 512, (e + 1) * 512)
                        ms = slice(mc * 128, (mc + 1) * 128)
                        po = spool[e].tile([128, 512], dt.float32, tag="s",
                                           name="po")
                        for h in range(HPC):
                            nc.tensor.matmul(po, lhsT=cun_t[(mac, h)][:, ms],
                                             rhs=woT[h][:, es],
                                             start=(h == 0), stop=(h == 1),
                                             skip_group_check=True)
                        if e == 0:
                            nc.vector.tensor_copy(out=ot[:, es], in_=po)
                        else:
                            nc.scalar.copy(out=ot[:, es], in_=po)
                    r0 = mac * MAC + mc * 128
                    nc.sync.dma_start(out=out_d.ap()[r0:r0 + 128, :], in_=ot)

    nc.compile()
    return nc


def kernel(**inputs):
    global LAST_RESULT
    import ml_dtypes
    from concourse.bass_utils import run_bass_kernel_spmd

    x = np.asarray(inputs["x"], dtype=np.float32).reshape(S, DIM)
    ln_g = np.asarray(inputs["ln_g"], dtype=np.float32)
    ln_b = np.asarray(inputs["ln_b"], dtype=np.float32)
    Wq = np.asarray(inputs["Wq"], dtype=np.float32)
    Wk = np.asarray(inputs["Wk"], dtype=np.float32)
    Wv = np.asarray(inputs["Wv"], dtype=np.float32)
    Wo = np.asarray(inputs["Wo"], dtype=np.float32)
    bq = np.asarray(inputs["bq"], dtype=np.float32)
    bk = np.asarray(inputs["bk"], dtype=np.float32)
    bv = np.asarray(inputs["bv"], dtype=np.float32)
    bo = np.asarray(inputs["bo"], dtype=np.float32)

    if "nc" not in _CACHE:
        _CACHE["nc"] = _build()
    nc = _CACHE["nc"]

    bf16 = ml_dtypes.bfloat16
    in_maps = []
    for c in range(N_CORES):
        sl = slice(c * DC, (c + 1) * DC)
        in_maps.append({
            "x": x,
            "wqT": np.ascontiguousarray((Wq[sl] * ln_g[None, :]).T).astype(bf16),
            "wkT": np.ascontiguousarray((Wk[sl] * ln_g[None, :]).T).astype(bf16),
            "wvT": np.ascontiguousarray((Wv[sl] * ln_g[None, :]).T).astype(bf16),
            "woT": np.ascontiguousarray(Wo[:, sl].T).astype(bf16),
            "bq": bq[sl] + Wq[sl] @ ln_b,
            "bk": bk[sl] + Wk[sl] @ ln_b,
            "bv": bv[sl] + Wv[sl] @ ln_b,
        })

    res = run_bass_kernel_spmd(nc, in_maps, list(range(N_CORES)))
    LAST_RESULT = res

    acc = res.results[0]["out"].astype(np.float32)
    for c in range(1, N_CORES):
        acc = acc + res.results[c]["out"].astype(np.float32)
    acc += bo[None, :]
    return acc.reshape(B, S, DIM)


# revision 46
# speedup vs baseline: 1.1059x; 1.1059x over previous
"""Trainium2 Bass kernel: LayerNorm -> MHA(16 heads, S=4096, D=1024) -> out-proj.

Sharding: tensor-parallel over heads. 8 cores x 2 heads each.
Each core computes LN(x) (replicated), q/k/v for its 2 heads (columns of
Wq/Wk/Wv), attention for those heads, and a partial output projection
(its 128 rows of Wo.T) in bf16. Host sums the 8 partials and adds bo.

v3 notes (what profiling taught us):
  - The chip power-manages the PE: sustained-dense phases run at an
    effective ~1.2 GHz (hardware duty cap k=4/8 in the NTFF 'ham' spans),
    light phases at 2.4 GHz. Dense attention streaming is therefore
    ~427ns per 512-col pass no matter the dtype (fp32r/bf16/fp8 all
    measured the same) -- so everything is 16-bit for margin, not speed.
  - The t-loop is kept minimal (8 matmuls + 2 exps per t); evacuation,
    softmax-sum reciprocal, out-projection and output DMA all live
    outside the dense loop where the clock is granted in full.
  - Softmax normalization is applied to out-projection ROWS (per-head
    1/sum scaling on PSUM evacuation: ACT does po0*r0, DVE does
    po1*r1 + that), which deletes v1's per-chunk transpose-normalize.
  - LN apply alternates DVE/ACT per tile to balance engine load.

Per-core layout (core c, heads 2c, 2c+1; d-slice = [128c, 128c+128)):
  phase 1: LN in [m,d] tiles -> PE-transpose -> hT [d,m] bf16;
           q/k = W.T @ hT + b -> bf16 [128d, S]; v -> transpose ->
           [t, (tc,head,65)] fp16 (ones col accumulates softmax sums)
  phase 2: scoresT[t,m] = kT.T @ qT per head (K=64, disjoint PE row
           tiles) -> exp (ACT, scale=1/32) -> fp16 w;
           ctx_u[65,m] += v_aug.T @ w accumulated over t in PSUM;
           per-mac: ctx_u -> sbuf bf16, sums -> transpose -> 1/sums.
  phase 3: po_h[m,e] = ctx_u_h.T @ woT_h; out = po0*r0 + po1*r1 (bf16)
           -> DMA per 128-row chunk.

LN gain g is folded into Wq/Wk/Wv columns host-side; LN bias b_ln is folded
into bq/bk/bv.  bo is added host-side after the cross-core reduction.
"""

import math
import os
from contextlib import ExitStack

import numpy as np

B, S, DIM, H = 1, 4096, 1024, 16
HD = DIM // H            # 64
N_CORES = 8
HPC = H // N_CORES       # 2 heads per core
DC = HPC * HD            # 128 dims per core
MB = 512                 # phase-1 m-block
N_MB = S // MB           # 8
MAC = 1024               # phase-2 m-macro
N_MAC = S // MAC         # 4
TC = S // 128            # 32 t-chunks
SCALE = 1.0 / math.sqrt(DIM)

FP8 = os.environ.get("FP8", "0") == "1"   # fp8 q/k: no speed gain measured,
                                          # costs error margin; off by default
_CACHE = {}
LAST_RESULT = None       # BassKernelResults of the most recent run (for test.py)


def _build():
    import concourse.bacc as bacc
    import concourse.tile as tile
    import concourse.mybir as mybir
    from concourse.masks import make_identity

    dt = mybir.dt
    AF = mybir.ActivationFunctionType
    ALU = mybir.AluOpType

    qk_dt = dt.float8e4 if FP8 else dt.bfloat16

    nc = bacc.Bacc("TRN2", target_bir_lowering=False, debug=False,
                   num_devices=N_CORES)

    x_d = nc.dram_tensor("x", [S, DIM], dt.float32, kind="ExternalInput")
    wqT_d = nc.dram_tensor("wqT", [DIM, DC], dt.bfloat16, kind="ExternalInput")
    wkT_d = nc.dram_tensor("wkT", [DIM, DC], dt.bfloat16, kind="ExternalInput")
    wvT_d = nc.dram_tensor("wvT", [DIM, DC], dt.bfloat16, kind="ExternalInput")
    woT_d = nc.dram_tensor("woT", [DC, DIM], dt.bfloat16, kind="ExternalInput")
    bq_d = nc.dram_tensor("bq", [DC], dt.float32, kind="ExternalInput")
    bk_d = nc.dram_tensor("bk", [DC], dt.float32, kind="ExternalInput")
    bv_d = nc.dram_tensor("bv", [DC], dt.float32, kind="ExternalInput")
    out_d = nc.dram_tensor("out", [S, DIM], dt.bfloat16, kind="ExternalOutput")

    with tile.TileContext(nc) as tc, ExitStack() as top:
        persist = top.enter_context(tc.tile_pool(name="persist", bufs=1))

        ident = persist.tile([128, 128], dt.float32)
        ident_bf = persist.tile([128, 128], dt.bfloat16)
        eps_t = persist.tile([128, 1], dt.float32)
        ones_bf = persist.tile([128, HD], dt.bfloat16)

        wT = {n: persist.tile([128, DIM // 128, DC], dt.bfloat16,
                              tag=f"w{n}T", name=f"w{n}T")
              for n in ("q", "k", "v")}
        woT = {h: persist.tile([HD, DIM], dt.bfloat16, tag=f"woT{h}",
                               name=f"woT{h}")
               for h in range(HPC)}
        bias = {n: persist.tile([DC, 1], dt.float32, tag=f"b{n}", name=f"b{n}")
                for n in ("q", "k", "v")}

        qf = persist.tile([DC, S], qk_dt, tag="qf")
        kf = persist.tile([DC, S], qk_dt, tag="kf")
        # v with an appended ones-column per head: [t-part, tc, head, HD+1]
        v_all = persist.tile([128, TC, HPC, HD + 1], dt.float16)

        # ---------------- phase 1: LN + QKV projections ----------------
        with ExitStack() as p1:
            xpool = p1.enter_context(tc.tile_pool(name="xp", bufs=10))
            hpool = p1.enter_context(tc.tile_pool(name="hp", bufs=5))
            hTpool = p1.enter_context(tc.tile_pool(name="hTp", bufs=3))
            stat = p1.enter_context(tc.tile_pool(name="stat", bufs=8))
            vsb = p1.enter_context(tc.tile_pool(name="vsb", bufs=2))
            ps_t = p1.enter_context(tc.tile_pool(name="ps_t", bufs=2, space="PSUM"))
            ps_p = p1.enter_context(tc.tile_pool(name="ps_p", bufs=4, space="PSUM"))
            ps_v = p1.enter_context(tc.tile_pool(name="ps_v", bufs=2, space="PSUM"))

            # first two m-blocks' x tiles: DMA queued before anything else,
            # split in column halves so each tile arrives via two queues
            def load_x(r0, name=None):
                xt = xpool.tile([128, DIM], dt.float32, tag="x",
                                name=name or "xt")
                for c in range(2):
                    cs = slice(c * 512, (c + 1) * 512)
                    nc.sync.dma_start(out=xt[:, cs],
                                      in_=x_d.ap()[r0:r0 + 128, cs])
                return xt

            first_x = {}
            for mb in range(2):
                for j in range(MB // 128):
                    first_x[(mb, j)] = load_x(mb * MB + j * 128, name="xt0")

            make_identity(nc, ident)
            nc.vector.tensor_copy(out=ident_bf, in_=ident)
            nc.vector.memset(eps_t, 1e-5)
            nc.vector.memset(ones_bf, 1.0)
            nc.vector.memset(v_all, 1.0)
            for n, d in (("q", wqT_d), ("k", wkT_d), ("v", wvT_d)):
                nc.sync.dma_start(out=wT[n], in_=d.ap().rearrange(
                    "(c p) n -> p c n", p=128))
            for h in range(HPC):
                nc.sync.dma_start(out=woT[h],
                                  in_=woT_d.ap()[h * HD:(h + 1) * HD, :])
            for n, d in (("q", bq_d), ("k", bk_d), ("v", bv_d)):
                nc.sync.dma_start(out=bias[n], in_=d.ap()[:, None])

            for mb in range(N_MB):
                hs = []
                for j in range(MB // 128):
                    if (mb, j) in first_x:
                        xt = first_x[(mb, j)]
                    else:
                        xt = load_x(mb * MB + j * 128)
                    st = stat.tile([128, 2, nc.vector.BN_STATS_DIM],
                                   dt.float32, tag="st")
                    xg = xt[:].rearrange("p (s f) -> p s f", s=2)
                    for sg in range(2):
                        nc.vector.bn_stats(out=st[:, sg, :], in_=xg[:, sg, :])
                    mv = stat.tile([128, 2], dt.float32, tag="mv")
                    nc.vector.bn_aggr(out=mv, in_=st)
                    std = stat.tile([128, 1], dt.float32, tag="sd")
                    nc.scalar.activation(out=std, in_=mv[:, 1:2], func=AF.Sqrt,
                                         bias=eps_t, scale=1.0)
                    rstd = stat.tile([128, 1], dt.float32, tag="rs")
                    nc.vector.reciprocal(out=rstd, in_=std)
                    ht = hpool.tile([128, DIM], dt.bfloat16, tag="h")
                    if j % 2 == 0:
                        # ACT path: h = Copy(rstd*x + (-mu*rstd))
                        nb = stat.tile([128, 1], dt.float32, tag="nb")
                        nc.vector.tensor_scalar(
                            out=nb, in0=mv[:, 0:1], scalar1=rstd,
                            scalar2=-1.0, op0=ALU.mult, op1=ALU.mult)
                        nc.scalar.activation(out=ht, in_=xt, func=AF.Identity,
                                             bias=nb, scale=rstd)
                    else:
                        nc.vector.tensor_scalar(
                            out=ht, in0=xt, scalar1=mv[:, 0:1],
                            scalar2=rstd, op0=ALU.subtract, op1=ALU.mult)
                    hs.append(ht)

                # transpose h -> hT  [128d, dc, 512m]  (bf16)
                hT = hTpool.tile([128, DIM // 128, MB], dt.bfloat16, tag="hT")
                for dc in range(DIM // 128):
                    pt = ps_t.tile([128, MB], dt.bfloat16, tag="pt")
                    for j in range(MB // 128):
                        nc.tensor.transpose(
                            pt[:, j * 128:(j + 1) * 128],
                            hs[j][:, dc * 128:(dc + 1) * 128], ident_bf)
                    nc.scalar.copy(out=hT[:, dc, :], in_=pt)

                mbs = slice(mb * MB, (mb + 1) * MB)
                for name in ("q", "k", "v"):
                    pp = ps_p.tile([128, MB], dt.float32, tag="pp")
                    for dc in range(DIM // 128):
                        nc.tensor.matmul(pp, lhsT=wT[name][:, dc, :],
                                         rhs=hT[:, dc, :],
                                         start=(dc == 0), stop=(dc == 7))
                    if name != "v":
                        dest = qf if name == "q" else kf
                        nc.vector.tensor_scalar(
                            out=dest[:, mbs], in0=pp,
                            scalar1=bias[name], scalar2=None, op0=ALU.add)
                    else:
                        vT = vsb.tile([128, MB], dt.bfloat16, tag="vT")
                        nc.vector.tensor_scalar(
                            out=vT, in0=pp, scalar1=bias[name], scalar2=None,
                            op0=ALU.add)
                        pv = ps_v.tile([128, MB], dt.bfloat16, tag="pv")
                        for j in range(MB // 128):
                            nc.tensor.transpose(
                                pv[:, j * 128:(j + 1) * 128],
                                vT[:, j * 128:(j + 1) * 128], ident_bf)
                        for j in range(MB // 128):
                            tc_j = mb * (MB // 128) + j
                            src = pv[:, j * 128:(j + 1) * 128].rearrange(
                                "p (h e) -> p h e", h=HPC)
                            nc.vector.tensor_copy(
                                out=v_all[:, tc_j, :, 0:HD], in_=src)

        # ---------------- phase 2: attention (dense t-loop) ----------------
        # scores/exp use SEPARATE 512-col psum tiles per half (tags sa/sb):
        # whole-tile dependency tracking then lets exp(j) start right after
        # its own scores half, and AV(j) right after its own exp half.
        with ExitStack() as p2:
            spool = {j: p2.enter_context(
                tc.tile_pool(name=f"sp{j}", bufs=2, space="PSUM"))
                for j in range(MAC // 512)}
            cpool = p2.enter_context(tc.tile_pool(name="cp", bufs=2, space="PSUM"))
            wpool = {j: p2.enter_context(tc.tile_pool(name=f"wp{j}", bufs=4))
                     for j in range(MAC // 512)}
            upool = p2.enter_context(tc.tile_pool(name="up", bufs=3))
            rpool = p2.enter_context(tc.tile_pool(name="rp", bufs=2))
            opool = p2.enter_context(tc.tile_pool(name="op", bufs=3))

            cu_t = {}
            cun_t = {}
            pcu_t = {}
            ot_t = {}

            def evac_slot(mac, slot):
                """Deferred mac-boundary work, one light piece per t-slot of
                the following mac: 0 -> psum->sbuf ctx_u copies; 1/2 -> per
                head: r = 1/sums row, rbc = ones^T r (PE broadcast down 64
                partitions), ctx_n = ctx_u * rbc. Normalizing ctx along the
                free (m) dim here removes both the per-chunk sums transposes
                and any per-partition scaling in the out-projection."""
                if slot == 0:
                    for h in range(HPC):
                        cu = upool.tile([HD + 1, MAC], dt.bfloat16, tag="cu",
                                        name=f"cu{h}")
                        nc.vector.tensor_copy(out=cu, in_=pcu_t[(mac, h)])
                        cu_t[(mac, h)] = cu
                elif slot in (1, 2):
                    h = slot - 1
                    cu = cu_t[(mac, h)]
                    rr = rpool.tile([128, MAC], dt.bfloat16, tag="rr",
                                    name="rr")
                    with nc.allow_low_precision(
                            reason="softmax sums ~4e3; bf16 recip = 0.4%"):
                        nc.vector.reciprocal(out=rr[HD:HD + 1, :],
                                             in_=cu[HD:HD + 1, :])
                    rbc = cpool.tile([HD, MAC], dt.float32, tag="pc",
                                     name="rbc")
                    for j in range(MAC // 512):
                        js = slice(j * 512, (j + 1) * 512)
                        nc.tensor.matmul(rbc[:, js],
                                         lhsT=ones_bf[HD:HD + 1, :],
                                         rhs=rr[HD:HD + 1, js],
                                         start=True, stop=True)
                    cun = upool.tile([HD, MAC], dt.bfloat16, tag="cun",
                                     name=f"cun{h}", bufs=4)
                    nc.vector.tensor_tensor(out=cun, in0=cu[0:HD, :],
                                            in1=rbc, op=ALU.mult)
                    cun_t[(mac, h)] = cun

            for mac in range(N_MAC):
                m0 = mac * MAC
                for h in range(HPC):
                    pcu_t[(mac, h)] = cpool.tile([HD + 1, MAC], dt.float32,
                                                 tag="pc", name=f"pcu{h}")
                for t in range(TC):
                    ws = {}
                    for h in range(HPC):
                        hd0 = h * HD
                        for j in range(MAC // 512):
                            ps = spool[j].tile([128, 512], dt.float32,
                                               tag="s", name=f"ps{h}{j}")
                            nc.tensor.matmul(
                                ps,
                                lhsT=kf[hd0:hd0 + HD, t * 128:(t + 1) * 128],
                                rhs=qf[hd0:hd0 + HD,
                                       m0 + j * 512:m0 + (j + 1) * 512],
                                start=True, stop=True,
                                tile_position=(hd0, 0))
                            w = wpool[j].tile([128, 512], dt.float16,
                                              tag="w", name=f"w{h}{j}")
                            nc.scalar.activation(out=w, in_=ps,
                                                 func=AF.Exp, scale=SCALE)
                            ws[(h, j)] = w
                    for h in range(HPC):
                        for j in range(MAC // 512):
                            js = slice(j * 512, (j + 1) * 512)
                            nc.tensor.matmul(
                                pcu_t[(mac, h)][:, js],
                                lhsT=v_all[:, t, h, :],
                                rhs=ws[(h, j)],
                                start=(t == 0), stop=(t == TC - 1),
                                skip_group_check=True)
                    if mac > 0:
                        evac_slot(mac - 1, t)

            # ---------------- phase 3: out-projection tail ----------------
            # ctx is pre-normalized, so both heads accumulate into one PSUM
            # tile and the evacuation is a plain copy (alternating DVE/ACT).
            for slot in range(3):
                evac_slot(N_MAC - 1, slot)
            for mac in range(N_MAC):
                for mc in range(MAC // 128):
                    ot = opool.tile([128, DIM], dt.bfloat16, tag="ot",
                                    name="ot")
                    for e in range(DIM // 512):
                        es = slice(e * 512, (e + 1) * 512)
                        ms = slice(mc * 128, (mc + 1) * 128)
                        po = spool[e].tile([128, 512], dt.float32, tag="s",
                                           name="po")
                        for h in range(HPC):
                            nc.tensor.matmul(po, lhsT=cun_t[(mac, h)][:, ms],
                                             rhs=woT[h][:, es],
                                             start=(h == 0), stop=(h == 1),
                                             skip_group_check=True)
                        if e == 0:
                            nc.vector.tensor_copy(out=ot[:, es], in_=po)
                        else:
                            nc.scalar.copy(out=ot[:, es], in_=po)
                    r0 = mac * MAC + mc * 128
                    nc.sync.dma_start(out=out_d.ap()[r0:r0 + 128, :], in_=ot)

    nc.compile()
    return nc


def kernel(**inputs):
    global LAST_RESULT
    import ml_dtypes
    from concourse.bass_utils import run_bass_kernel_spmd

    x = np.asarray(inputs["x"], dtype=np.float32).reshape(S, DIM)
    ln_g = np.asarray(inputs["ln_g"], dtype=np.float32)
    ln_b = np.asarray(inputs["ln_b"], dtype=np.float32)
    Wq = np.asarray(inputs["Wq"], dtype=np.float32)
    Wk = np.asarray(inputs["Wk"], dtype=np.float32)
    Wv = np.asarray(inputs["Wv"], dtype=np.float32)
    Wo = np.asarray(inputs["Wo"], dtype=np.float32)
    bq = np.asarray(inputs["bq"], dtype=np.float32)
    bk = np.asarray(inputs["bk"], dtype=np.float32)
    bv = np.asarray(inputs["bv"], dtype=np.float32)
    bo = np.asarray(inputs["bo"], dtype=np.float32)

    if "nc" not in _CACHE:
        _CACHE["nc"] = _build()
    nc = _CACHE["nc"]

    bf16 = ml_dtypes.bfloat16
    in_maps = []
    for c in range(N_CORES):
        sl = slice(c * DC, (c + 1) * DC)
        in_maps.append({
            "x": x,
            "wqT": np.ascontiguousarray((Wq[sl] * ln_g[None, :]).T).astype(bf16),
            "wkT": np.ascontiguousarray((Wk[sl] * ln_g[None, :]).T).astype(bf16),
            "wvT": np.ascontiguousarray((Wv[sl] * ln_g[None, :]).T).astype(bf16),
            "woT": np.ascontiguousarray(Wo[:, sl].T).astype(bf16),
            "bq": bq[sl] + Wq[sl] @ ln_b,
            "bk": bk[sl] + Wk[sl] @ ln_b,
            "bv": bv[sl] + Wv[sl] @ ln_b,
        })

    res = run_bass_kernel_spmd(nc, in_maps, list(range(N_CORES)))
    LAST_RESULT = res

    acc = res.results[0]["out"].astype(np.float32)
    for c in range(1, N_CORES):
        acc = acc + res.results[c]["out"].astype(np.float32)
    acc += bo[None, :]
    return acc.reshape(B, S, DIM)


# revision 59
# speedup vs baseline: 1.1179x; 1.0109x over previous
"""Trainium2 Bass kernel: LayerNorm -> MHA(16 heads, S=4096, D=1024) -> out-proj.

Sharding: tensor-parallel over heads. 8 cores x 2 heads each.
Each core computes LN(x) (replicated), q/k/v for its 2 heads (columns of
Wq/Wk/Wv), attention for those heads, and a partial output projection
(its 128 rows of Wo.T) in bf16. Host sums the 8 partials and adds bo.

v3 notes (what profiling taught us):
  - The chip power-manages the PE: sustained-dense phases run at an
    effective ~1.2 GHz (hardware duty cap k=4/8 in the NTFF 'ham' spans),
    light phases at 2.4 GHz. Dense attention streaming is therefore
    ~427ns per 512-col pass no matter the dtype (fp32r/bf16/fp8 all
    measured the same) -- so everything is 16-bit for margin, not speed.
  - The t-loop is kept minimal (8 matmuls + 2 exps per t); evacuation,
    softmax-sum reciprocal, out-projection and output DMA all live
    outside the dense loop where the clock is granted in full.
  - Softmax normalization is applied to out-projection ROWS (per-head
    1/sum scaling on PSUM evacuation: ACT does po0*r0, DVE does
    po1*r1 + that), which deletes v1's per-chunk transpose-normalize.
  - LN apply alternates DVE/ACT per tile to balance engine load.

Per-core layout (core c, heads 2c, 2c+1; d-slice = [128c, 128c+128)):
  phase 1: LN in [m,d] tiles -> PE-transpose -> hT [d,m] bf16;
           q/k = W.T @ hT + b -> bf16 [128d, S]; v -> transpose ->
           [t, (tc,head,65)] fp16 (ones col accumulates softmax sums)
  phase 2: scoresT[t,m] = kT.T @ qT per head (K=64, disjoint PE row
           tiles) -> exp (ACT, scale=1/32) -> fp16 w;
           ctx_u[65,m] += v_aug.T @ w accumulated over t in PSUM;
           per-mac: ctx_u -> sbuf bf16, sums -> transpose -> 1/sums.
  phase 3: po_h[m,e] = ctx_u_h.T @ woT_h; out = po0*r0 + po1*r1 (bf16)
           -> DMA per 128-row chunk.

LN gain g is folded into Wq/Wk/Wv columns host-side; LN bias b_ln is folded
into bq/bk/bv.  bo is added host-side after the cross-core reduction.
"""

import math
import os
from contextlib import ExitStack

import numpy as np

B, S, DIM, H = 1, 4096, 1024, 16
HD = DIM // H            # 64
N_CORES = 8
HPC = H // N_CORES       # 2 heads per core
DC = HPC * HD            # 128 dims per core
MB = 512                 # phase-1 m-block
N_MB = S // MB           # 8
MAC = 1024               # phase-2 m-macro
N_MAC = S // MAC         # 4
TC = S // 128            # 32 t-chunks
SCALE = 1.0 / math.sqrt(DIM)

FP8 = os.environ.get("FP8", "0") == "1"   # fp8 q/k: no speed gain measured,
                                          # costs error margin; off by default
_CACHE = {}
LAST_RESULT = None       # BassKernelResults of the most recent run (for test.py)


def _build():
    import concourse.bacc as bacc
    import concourse.tile as tile
    import concourse.mybir as mybir
    from concourse.masks import make_identity

    dt = mybir.dt
    AF = mybir.ActivationFunctionType
    ALU = mybir.AluOpType

    qk_dt = dt.float8e4 if FP8 else dt.bfloat16

    nc = bacc.Bacc("TRN2", target_bir_lowering=False, debug=False,
                   num_devices=N_CORES)

    x_d = nc.dram_tensor("x", [S, DIM], dt.float32, kind="ExternalInput")
    wqT_d = nc.dram_tensor("wqT", [DIM, DC], dt.bfloat16, kind="ExternalInput")
    wkT_d = nc.dram_tensor("wkT", [DIM, DC], dt.bfloat16, kind="ExternalInput")
    wvT_d = nc.dram_tensor("wvT", [DIM, DC], dt.bfloat16, kind="ExternalInput")
    woT_d = nc.dram_tensor("woT", [DC, DIM], dt.bfloat16, kind="ExternalInput")
    bq_d = nc.dram_tensor("bq", [DC], dt.float32, kind="ExternalInput")
    bk_d = nc.dram_tensor("bk", [DC], dt.float32, kind="ExternalInput")
    bv_d = nc.dram_tensor("bv", [DC], dt.float32, kind="ExternalInput")
    out_d = nc.dram_tensor("out", [S, DIM], dt.bfloat16, kind="ExternalOutput")

    with tile.TileContext(nc) as tc, ExitStack() as top:
        persist = top.enter_context(tc.tile_pool(name="persist", bufs=1))

        ident = persist.tile([128, 128], dt.float32)
        ident_bf = persist.tile([128, 128], dt.bfloat16)
        eps_t = persist.tile([128, 1], dt.float32)
        ones_bf = persist.tile([128, HD], dt.bfloat16)

        wT = {n: persist.tile([128, DIM // 128, DC], dt.bfloat16,
                              tag=f"w{n}T", name=f"w{n}T")
              for n in ("q", "k", "v")}
        woT = persist.tile([DC, DIM], dt.bfloat16)
        bias = {n: persist.tile([DC, 1], dt.float32, tag=f"b{n}", name=f"b{n}")
                for n in ("q", "k", "v")}

        qf = persist.tile([DC, S], qk_dt, tag="qf")
        kf = persist.tile([DC, S], qk_dt, tag="kf")
        # v with an appended ones-column per head: [t-part, tc, head, HD+1]
        v_all = persist.tile([128, TC, HPC, HD + 1], dt.float16)

        # ---------------- phase 1: LN + QKV projections ----------------
        with ExitStack() as p1:
            xpool = p1.enter_context(tc.tile_pool(name="xp", bufs=10))
            hpool = p1.enter_context(tc.tile_pool(name="hp", bufs=5))
            hTpool = p1.enter_context(tc.tile_pool(name="hTp", bufs=3))
            stat = p1.enter_context(tc.tile_pool(name="stat", bufs=8))
            vsb = p1.enter_context(tc.tile_pool(name="vsb", bufs=2))
            ps_t = p1.enter_context(tc.tile_pool(name="ps_t", bufs=2, space="PSUM"))
            ps_p = p1.enter_context(tc.tile_pool(name="ps_p", bufs=4, space="PSUM"))
            ps_v = p1.enter_context(tc.tile_pool(name="ps_v", bufs=2, space="PSUM"))

            # first two m-blocks' x tiles: DMA queued before anything else,
            # split in column halves so each tile arrives via two queues
            def load_x(r0, name=None):
                xt = xpool.tile([128, DIM], dt.float32, tag="x",
                                name=name or "xt")
                for c in range(2):
                    cs = slice(c * 512, (c + 1) * 512)
                    nc.sync.dma_start(out=xt[:, cs],
                                      in_=x_d.ap()[r0:r0 + 128, cs])
                return xt

            first_x = {}
            for mb in range(2):
                for j in range(MB // 128):
                    first_x[(mb, j)] = load_x(mb * MB + j * 128, name="xt0")

            make_identity(nc, ident)
            nc.vector.tensor_copy(out=ident_bf, in_=ident)
            nc.vector.memset(eps_t, 1e-5)
            nc.vector.memset(ones_bf, 1.0)
            nc.vector.memset(v_all, 1.0)
            for n, d in (("q", wqT_d), ("k", wkT_d), ("v", wvT_d)):
                nc.sync.dma_start(out=wT[n], in_=d.ap().rearrange(
                    "(c p) n -> p c n", p=128))
            nc.sync.dma_start(out=woT, in_=woT_d.ap())
            for n, d in (("q", bq_d), ("k", bk_d), ("v", bv_d)):
                nc.sync.dma_start(out=bias[n], in_=d.ap()[:, None])

            for mb in range(N_MB):
                hs = []
                for j in range(MB // 128):
                    if (mb, j) in first_x:
                        xt = first_x[(mb, j)]
                    else:
                        xt = load_x(mb * MB + j * 128)
                    st = stat.tile([128, 2, nc.vector.BN_STATS_DIM],
                                   dt.float32, tag="st")
                    xg = xt[:].rearrange("p (s f) -> p s f", s=2)
                    for sg in range(2):
                        nc.vector.bn_stats(out=st[:, sg, :], in_=xg[:, sg, :])
                    mv = stat.tile([128, 2], dt.float32, tag="mv")
                    nc.vector.bn_aggr(out=mv, in_=st)
                    std = stat.tile([128, 1], dt.float32, tag="sd")
                    nc.scalar.activation(out=std, in_=mv[:, 1:2], func=AF.Sqrt,
                                         bias=eps_t, scale=1.0)
                    rstd = stat.tile([128, 1], dt.float32, tag="rs")
                    nc.vector.reciprocal(out=rstd, in_=std)
                    ht = hpool.tile([128, DIM], dt.bfloat16, tag="h")
                    if j % 2 == 0:
                        # ACT path: h = Copy(rstd*x + (-mu*rstd))
                        nb = stat.tile([128, 1], dt.float32, tag="nb")
                        nc.vector.tensor_scalar(
                            out=nb, in0=mv[:, 0:1], scalar1=rstd,
                            scalar2=-1.0, op0=ALU.mult, op1=ALU.mult)
                        nc.scalar.activation(out=ht, in_=xt, func=AF.Identity,
                                             bias=nb, scale=rstd)
                    else:
                        nc.vector.tensor_scalar(
                            out=ht, in0=xt, scalar1=mv[:, 0:1],
                            scalar2=rstd, op0=ALU.subtract, op1=ALU.mult)
                    hs.append(ht)

                # transpose h -> hT  [128d, dc, 512m]  (bf16)
                hT = hTpool.tile([128, DIM // 128, MB], dt.bfloat16, tag="hT")
                for dc in range(DIM // 128):
                    pt = ps_t.tile([128, MB], dt.bfloat16, tag="pt")
                    for j in range(MB // 128):
                        nc.tensor.transpose(
                            pt[:, j * 128:(j + 1) * 128],
                            hs[j][:, dc * 128:(dc + 1) * 128], ident_bf)
                    nc.scalar.copy(out=hT[:, dc, :], in_=pt)

                mbs = slice(mb * MB, (mb + 1) * MB)
                for name in ("q", "k", "v"):
                    pp = ps_p.tile([128, MB], dt.float32, tag="pp")
                    for dc in range(DIM // 128):
                        nc.tensor.matmul(pp, lhsT=wT[name][:, dc, :],
                                         rhs=hT[:, dc, :],
                                         start=(dc == 0), stop=(dc == 7))
                    if name != "v":
                        dest = qf if name == "q" else kf
                        nc.vector.tensor_scalar(
                            out=dest[:, mbs], in0=pp,
                            scalar1=bias[name], scalar2=None, op0=ALU.add)
                    else:
                        vT = vsb.tile([128, MB], dt.bfloat16, tag="vT")
                        nc.vector.tensor_scalar(
                            out=vT, in0=pp, scalar1=bias[name], scalar2=None,
                            op0=ALU.add)
                        pv = ps_v.tile([128, MB], dt.bfloat16, tag="pv")
                        for j in range(MB // 128):
                            nc.tensor.transpose(
                                pv[:, j * 128:(j + 1) * 128],
                                vT[:, j * 128:(j + 1) * 128], ident_bf)
                        for j in range(MB // 128):
                            tc_j = mb * (MB // 128) + j
                            src = pv[:, j * 128:(j + 1) * 128].rearrange(
                                "p (h e) -> p h e", h=HPC)
                            nc.vector.tensor_copy(
                                out=v_all[:, tc_j, :, 0:HD], in_=src)

        # ---------------- phase 2: attention (dense t-loop) ----------------
        # scores/exp use SEPARATE 512-col psum tiles per half (tags sa/sb):
        # whole-tile dependency tracking then lets exp(j) start right after
        # its own scores half, and AV(j) right after its own exp half.
        with ExitStack() as p2:
            spool = {j: p2.enter_context(
                tc.tile_pool(name=f"sp{j}", bufs=2, space="PSUM"))
                for j in range(MAC // 512)}
            cpool = p2.enter_context(tc.tile_pool(name="cp", bufs=2, space="PSUM"))
            wpool = {j: p2.enter_context(tc.tile_pool(name=f"wp{j}", bufs=4))
                     for j in range(MAC // 512)}
            upool = p2.enter_context(tc.tile_pool(name="up", bufs=3))
            rpool = p2.enter_context(tc.tile_pool(name="rp", bufs=2))
            opool = p2.enter_context(tc.tile_pool(name="op", bufs=3))

            cu_t = {}
            cun_t = {}
            rbc_t = {}
            pcu_t = {}

            def evac_slot(mac, slot):
                """Deferred mac-boundary work, one light piece per t-slot of
                the following mac: 0 -> psum->sbuf ctx_u copies; 1/2 -> per
                head: r = 1/sums row, rbc = ones^T r (PE broadcast down 64
                partitions), ctx_n = ctx_u * rbc. Normalizing ctx along the
                free (m) dim here removes both the per-chunk sums transposes
                and any per-partition scaling in the out-projection."""
                if slot == 0:
                    for h in range(HPC):
                        cu = upool.tile([HD + 1, MAC], dt.bfloat16, tag="cu",
                                        name=f"cu{h}")
                        nc.vector.tensor_copy(out=cu, in_=pcu_t[(mac, h)])
                        cu_t[(mac, h)] = cu
                elif slot in (1, 2):
                    h = slot - 1
                    cu = cu_t[(mac, h)]
                    rr = rpool.tile([128, MAC], dt.bfloat16, tag="rr",
                                    name="rr")
                    with nc.allow_low_precision(
                            reason="softmax sums ~4e3; bf16 recip = 0.4%"):
                        nc.vector.reciprocal(out=rr[HD:HD + 1, :],
                                             in_=cu[HD:HD + 1, :])
                    rbc = cpool.tile([HD, MAC], dt.float32, tag="pc",
                                     name="rbc")
                    for j in range(MAC // 512):
                        js = slice(j * 512, (j + 1) * 512)
                        nc.tensor.matmul(rbc[:, js],
                                         lhsT=ones_bf[HD:HD + 1, :],
                                         rhs=rr[HD:HD + 1, js],
                                         start=True, stop=True)
                    if h == 0:
                        cun_t[mac] = upool.tile([128, MAC], dt.bfloat16,
                                                tag="cun", name="cun", bufs=4)
                        nc.vector.tensor_tensor(out=cun_t[mac][0:HD, :],
                                                in0=cu[0:HD, :],
                                                in1=rbc, op=ALU.mult)
                    else:
                        c1 = upool.tile([HD, MAC], dt.bfloat16, tag="c1",
                                        name="c1", bufs=2)
                        nc.vector.tensor_tensor(out=c1, in0=cu[0:HD, :],
                                                in1=rbc, op=ALU.mult)
                        cun_t[(mac, 1)] = c1
                elif slot == 3:
                    # PE-shift h1's normalized ctx to partitions [64:128) so
                    # the out-projection is one K=128 matmul per chunk
                    c1 = cun_t[(mac, 1)]
                    psh = cpool.tile([128, MAC], dt.float32, tag="pc",
                                     name="psh")
                    for j in range(MAC // 512):
                        js = slice(j * 512, (j + 1) * 512)
                        nc.tensor.matmul(psh[HD:128, js],
                                         lhsT=ident_bf[0:HD, 0:HD],
                                         rhs=c1[:, js],
                                         start=True, stop=True)
                    nc.vector.tensor_copy(out=cun_t[mac][HD:128, :],
                                          in_=psh[HD:128, :])

            for mac in range(N_MAC):
                m0 = mac * MAC
                for h in range(HPC):
                    pcu_t[(mac, h)] = cpool.tile([HD + 1, MAC], dt.float32,
                                                 tag="pc", name=f"pcu{h}")
                for t in range(TC):
                    ws = {}
                    for h in range(HPC):
                        hd0 = h * HD
                        for j in range(MAC // 512):
                            ps = spool[j].tile([128, 512], dt.float32,
                                               tag="s", name=f"ps{h}{j}")
                            nc.tensor.matmul(
                                ps,
                                lhsT=kf[hd0:hd0 + HD, t * 128:(t + 1) * 128],
                                rhs=qf[hd0:hd0 + HD,
                                       m0 + j * 512:m0 + (j + 1) * 512],
                                start=True, stop=True,
                                tile_position=(hd0, 0))
                            w = wpool[j].tile([128, 512], dt.float16,
                                              tag="w", name=f"w{h}{j}")
                            nc.scalar.activation(out=w, in_=ps,
                                                 func=AF.Exp, scale=SCALE)
                            ws[(h, j)] = w
                    for h in range(HPC):
                        for j in range(MAC // 512):
                            js = slice(j * 512, (j + 1) * 512)
                            nc.tensor.matmul(
                                pcu_t[(mac, h)][:, js],
                                lhsT=v_all[:, t, h, :],
                                rhs=ws[(h, j)],
                                start=(t == 0), stop=(t == TC - 1),
                                skip_group_check=True)
                    if mac > 0:
                        evac_slot(mac - 1, t)

            # ---------------- phase 3: out-projection tail ----------------
            # ctx is pre-normalized and both heads live on disjoint
            # partitions of one joint tile: out-proj is one K=128 matmul.
            for slot in range(4):
                evac_slot(N_MAC - 1, slot)
            for mac in range(N_MAC):
                for mc in range(MAC // 128):
                    ot = opool.tile([128, DIM], dt.bfloat16, tag="ot",
                                    name="ot")
                    for e in range(DIM // 512):
                        es = slice(e * 512, (e + 1) * 512)
                        ms = slice(mc * 128, (mc + 1) * 128)
                        po = spool[e].tile([128, 512], dt.float32, tag="s",
                                           name="po")
                        nc.tensor.matmul(po, lhsT=cun_t[mac][:, ms],
                                         rhs=woT[:, es],
                                         start=True, stop=True)
                        if e == 0:
                            nc.vector.tensor_copy(out=ot[:, es], in_=po)
                        else:
                            nc.scalar.copy(out=ot[:, es], in_=po)
                    r0 = mac * MAC + mc * 128
                    nc.sync.dma_start(out=out_d.ap()[r0:r0 + 128, :], in_=ot)

    nc.compile()
    return nc


def kernel(**inputs):
    global LAST_RESULT
    import ml_dtypes
    from concourse.bass_utils import run_bass_kernel_spmd

    x = np.asarray(inputs["x"], dtype=np.float32).reshape(S, DIM)
    ln_g = np.asarray(inputs["ln_g"], dtype=np.float32)
    ln_b = np.asarray(inputs["ln_b"], dtype=np.float32)
    Wq = np.asarray(inputs["Wq"], dtype=np.float32)
    Wk = np.asarray(inputs["Wk"], dtype=np.float32)
    Wv = np.asarray(inputs["Wv"], dtype=np.float32)
    Wo = np.asarray(inputs["Wo"], dtype=np.float32)
    bq = np.asarray(inputs["bq"], dtype=np.float32)
    bk = np.asarray(inputs["bk"], dtype=np.float32)
    bv = np.asarray(inputs["bv"], dtype=np.float32)
    bo = np.asarray(inputs["bo"], dtype=np.float32)

    if "nc" not in _CACHE:
        _CACHE["nc"] = _build()
    nc = _CACHE["nc"]

    bf16 = ml_dtypes.bfloat16
    in_maps = []
    for c in range(N_CORES):
        sl = slice(c * DC, (c + 1) * DC)
        in_maps.append({
            "x": x,
            "wqT": np.ascontiguousarray((Wq[sl] * ln_g[None, :]).T).astype(bf16),
            "wkT": np.ascontiguousarray((Wk[sl] * ln_g[None, :]).T).astype(bf16),
            "wvT": np.ascontiguousarray((Wv[sl] * ln_g[None, :]).T).astype(bf16),
            "woT": np.ascontiguousarray(Wo[:, sl].T).astype(bf16),
            "bq": bq[sl] + Wq[sl] @ ln_b,
            "bk": bk[sl] + Wk[sl] @ ln_b,
            "bv": bv[sl] + Wv[sl] @ ln_b,
        })

    res = run_bass_kernel_spmd(nc, in_maps, list(range(N_CORES)))
    LAST_RESULT = res

    acc = res.results[0]["out"].astype(np.float32)
    for c in range(1, N_CORES):
        acc = acc + res.results[c]["out"].astype(np.float32)
    acc += bo[None, :]
    return acc.reshape(B, S, DIM)


# revision 60
# speedup vs baseline: 1.1330x; 1.0135x over previous
"""Trainium2 Bass kernel: LayerNorm -> MHA(16 heads, S=4096, D=1024) -> out-proj.

Sharding: tensor-parallel over heads. 8 cores x 2 heads each.
Each core computes LN(x) (replicated), q/k/v for its 2 heads (columns of
Wq/Wk/Wv), attention for those heads, and a partial output projection
(its 128 rows of Wo.T) in bf16. Host sums the 8 partials and adds bo.

v3 notes (what profiling taught us):
  - The chip power-manages the PE: sustained-dense phases run at an
    effective ~1.2 GHz (hardware duty cap k=4/8 in the NTFF 'ham' spans),
    light phases at 2.4 GHz. Dense attention streaming is therefore
    ~427ns per 512-col pass no matter the dtype (fp32r/bf16/fp8 all
    measured the same) -- so everything is 16-bit for margin, not speed.
  - The t-loop is kept minimal (8 matmuls + 2 exps per t); evacuation,
    softmax-sum reciprocal, out-projection and output DMA all live
    outside the dense loop where the clock is granted in full.
  - Softmax normalization is applied to out-projection ROWS (per-head
    1/sum scaling on PSUM evacuation: ACT does po0*r0, DVE does
    po1*r1 + that), which deletes v1's per-chunk transpose-normalize.
  - LN apply alternates DVE/ACT per tile to balance engine load.

Per-core layout (core c, heads 2c, 2c+1; d-slice = [128c, 128c+128)):
  phase 1: LN in [m,d] tiles -> PE-transpose -> hT [d,m] bf16;
           q/k = W.T @ hT + b -> bf16 [128d, S]; v -> transpose ->
           [t, (tc,head,65)] fp16 (ones col accumulates softmax sums)
  phase 2: scoresT[t,m] = kT.T @ qT per head (K=64, disjoint PE row
           tiles) -> exp (ACT, scale=1/32) -> fp16 w;
           ctx_u[65,m] += v_aug.T @ w accumulated over t in PSUM;
           per-mac: ctx_u -> sbuf bf16, sums -> transpose -> 1/sums.
  phase 3: po_h[m,e] = ctx_u_h.T @ woT_h; out = po0*r0 + po1*r1 (bf16)
           -> DMA per 128-row chunk.

LN gain g is folded into Wq/Wk/Wv columns host-side; LN bias b_ln is folded
into bq/bk/bv.  bo is added host-side after the cross-core reduction.
"""

import math
import os
from contextlib import ExitStack

import numpy as np

B, S, DIM, H = 1, 4096, 1024, 16
HD = DIM // H            # 64
N_CORES = 8
HPC = H // N_CORES       # 2 heads per core
DC = HPC * HD            # 128 dims per core
MB = 512                 # phase-1 m-block
N_MB = S // MB           # 8
MAC = 1024               # phase-2 m-macro
N_MAC = S // MAC         # 4
TC = S // 128            # 32 t-chunks
SCALE = 1.0 / math.sqrt(DIM)

FP8 = os.environ.get("FP8", "0") == "1"   # fp8 q/k: no speed gain measured,
                                          # costs error margin; off by default
_CACHE = {}
LAST_RESULT = None       # BassKernelResults of the most recent run (for test.py)


def _build():
    import concourse.bacc as bacc
    import concourse.tile as tile
    import concourse.mybir as mybir
    from concourse.masks import make_identity

    dt = mybir.dt
    AF = mybir.ActivationFunctionType
    ALU = mybir.AluOpType

    qk_dt = dt.float8e4 if FP8 else dt.bfloat16

    nc = bacc.Bacc("TRN2", target_bir_lowering=False, debug=False,
                   num_devices=N_CORES)

    x_d = nc.dram_tensor("x", [S, DIM], dt.float32, kind="ExternalInput")
    wqT_d = nc.dram_tensor("wqT", [DIM, DC], dt.bfloat16, kind="ExternalInput")
    wkT_d = nc.dram_tensor("wkT", [DIM, DC], dt.bfloat16, kind="ExternalInput")
    wvT_d = nc.dram_tensor("wvT", [DIM, DC], dt.bfloat16, kind="ExternalInput")
    woT_d = nc.dram_tensor("woT", [DC, DIM], dt.bfloat16, kind="ExternalInput")
    bq_d = nc.dram_tensor("bq", [DC], dt.float32, kind="ExternalInput")
    bk_d = nc.dram_tensor("bk", [DC], dt.float32, kind="ExternalInput")
    bv_d = nc.dram_tensor("bv", [DC], dt.float32, kind="ExternalInput")
    out_d = nc.dram_tensor("out", [S, DIM], dt.bfloat16, kind="ExternalOutput")

    with tile.TileContext(nc) as tc, ExitStack() as top:
        persist = top.enter_context(tc.tile_pool(name="persist", bufs=1))

        ident = persist.tile([128, 128], dt.float32)
        ident_bf = persist.tile([128, 128], dt.bfloat16)
        eps_t = persist.tile([128, 1], dt.float32)
        ones_bf = persist.tile([128, HD], dt.bfloat16)

        wT = {n: persist.tile([128, DIM // 128, DC], dt.bfloat16,
                              tag=f"w{n}T", name=f"w{n}T")
              for n in ("q", "k", "v")}
        woT = persist.tile([DC, DIM], dt.bfloat16)
        bias = {n: persist.tile([DC, 1], dt.float32, tag=f"b{n}", name=f"b{n}")
                for n in ("q", "k", "v")}

        qf = persist.tile([DC, S], qk_dt, tag="qf")
        kf = persist.tile([DC, S], qk_dt, tag="kf")
        # v with an appended ones-column per head: [t-part, tc, head, HD+1]
        v_all = persist.tile([128, TC, HPC, HD + 1], dt.float16)

        # ---------------- phase 1: LN + QKV projections ----------------
        with ExitStack() as p1:
            xpool = p1.enter_context(tc.tile_pool(name="xp", bufs=10))
            hpool = p1.enter_context(tc.tile_pool(name="hp", bufs=5))
            hTpool = p1.enter_context(tc.tile_pool(name="hTp", bufs=3))
            stat = p1.enter_context(tc.tile_pool(name="stat", bufs=8))
            vsb = p1.enter_context(tc.tile_pool(name="vsb", bufs=2))
            ps_t = p1.enter_context(tc.tile_pool(name="ps_t", bufs=2, space="PSUM"))
            ps_p = p1.enter_context(tc.tile_pool(name="ps_p", bufs=4, space="PSUM"))
            ps_v = p1.enter_context(tc.tile_pool(name="ps_v", bufs=2, space="PSUM"))

            # first two m-blocks' x tiles: DMA queued before anything else,
            # split in column halves so each tile arrives via two queues
            def load_x(r0, name=None):
                xt = xpool.tile([128, DIM], dt.float32, tag="x",
                                name=name or "xt")
                for c in range(2):
                    cs = slice(c * 512, (c + 1) * 512)
                    nc.sync.dma_start(out=xt[:, cs],
                                      in_=x_d.ap()[r0:r0 + 128, cs])
                return xt

            first_x = {}
            for mb in range(2):
                for j in range(MB // 128):
                    first_x[(mb, j)] = load_x(mb * MB + j * 128, name="xt0")

            make_identity(nc, ident)
            nc.vector.tensor_copy(out=ident_bf, in_=ident)
            nc.vector.memset(eps_t, 1e-5)
            nc.vector.memset(ones_bf, 1.0)
            nc.vector.memset(v_all, 1.0)
            for n, d in (("q", wqT_d), ("k", wkT_d), ("v", wvT_d)):
                nc.sync.dma_start(out=wT[n], in_=d.ap().rearrange(
                    "(c p) n -> p c n", p=128))
            nc.sync.dma_start(out=woT, in_=woT_d.ap())
            for n, d in (("q", bq_d), ("k", bk_d), ("v", bv_d)):
                nc.sync.dma_start(out=bias[n], in_=d.ap()[:, None])

            for mb in range(N_MB):
                hs = []
                for j in range(MB // 128):
                    if (mb, j) in first_x:
                        xt = first_x[(mb, j)]
                    else:
                        xt = load_x(mb * MB + j * 128)
                    st = stat.tile([128, 2, nc.vector.BN_STATS_DIM],
                                   dt.float32, tag="st")
                    xg = xt[:].rearrange("p (s f) -> p s f", s=2)
                    for sg in range(2):
                        nc.vector.bn_stats(out=st[:, sg, :], in_=xg[:, sg, :])
                    mv = stat.tile([128, 2], dt.float32, tag="mv")
                    nc.vector.bn_aggr(out=mv, in_=st)
                    std = stat.tile([128, 1], dt.float32, tag="sd")
                    nc.scalar.activation(out=std, in_=mv[:, 1:2], func=AF.Sqrt,
                                         bias=eps_t, scale=1.0)
                    rstd = stat.tile([128, 1], dt.float32, tag="rs")
                    nc.vector.reciprocal(out=rstd, in_=std)
                    ht = hpool.tile([128, DIM], dt.bfloat16, tag="h")
                    if j % 2 == 0:
                        # ACT path: h = Copy(rstd*x + (-mu*rstd))
                        nb = stat.tile([128, 1], dt.float32, tag="nb")
                        nc.vector.tensor_scalar(
                            out=nb, in0=mv[:, 0:1], scalar1=rstd,
                            scalar2=-1.0, op0=ALU.mult, op1=ALU.mult)
                        nc.scalar.activation(out=ht, in_=xt, func=AF.Identity,
                                             bias=nb, scale=rstd)
                    else:
                        nc.vector.tensor_scalar(
                            out=ht, in0=xt, scalar1=mv[:, 0:1],
                            scalar2=rstd, op0=ALU.subtract, op1=ALU.mult)
                    hs.append(ht)

                # transpose h -> hT  [128d, dc, 512m]  (bf16)
                hT = hTpool.tile([128, DIM // 128, MB], dt.bfloat16, tag="hT")
                for dc in range(DIM // 128):
                    pt = ps_t.tile([128, MB], dt.bfloat16, tag="pt")
                    for j in range(MB // 128):
                        nc.tensor.transpose(
                            pt[:, j * 128:(j + 1) * 128],
                            hs[j][:, dc * 128:(dc + 1) * 128], ident_bf)
                    nc.scalar.copy(out=hT[:, dc, :], in_=pt)

                mbs = slice(mb * MB, (mb + 1) * MB)
                for name in ("q", "k", "v"):
                    pp = ps_p.tile([128, MB], dt.float32, tag="pp")
                    for dc in range(DIM // 128):
                        nc.tensor.matmul(pp, lhsT=wT[name][:, dc, :],
                                         rhs=hT[:, dc, :],
                                         start=(dc == 0), stop=(dc == 7))
                    if name != "v":
                        dest = qf if name == "q" else kf
                        nc.vector.tensor_scalar(
                            out=dest[:, mbs], in0=pp,
                            scalar1=bias[name], scalar2=None, op0=ALU.add)
                    else:
                        vT = vsb.tile([128, MB], dt.bfloat16, tag="vT")
                        nc.vector.tensor_scalar(
                            out=vT, in0=pp, scalar1=bias[name], scalar2=None,
                            op0=ALU.add)
                        pv = ps_v.tile([128, MB], dt.bfloat16, tag="pv")
                        for j in range(MB // 128):
                            nc.tensor.transpose(
                                pv[:, j * 128:(j + 1) * 128],
                                vT[:, j * 128:(j + 1) * 128], ident_bf)
                        for j in range(MB // 128):
                            tc_j = mb * (MB // 128) + j
                            src = pv[:, j * 128:(j + 1) * 128].rearrange(
                                "p (h e) -> p h e", h=HPC)
                            nc.vector.tensor_copy(
                                out=v_all[:, tc_j, :, 0:HD], in_=src)

        # ---------------- phase 2: attention (dense t-loop) ----------------
        # scores/exp use SEPARATE 512-col psum tiles per half (tags sa/sb):
        # whole-tile dependency tracking then lets exp(j) start right after
        # its own scores half, and AV(j) right after its own exp half.
        with ExitStack() as p2:
            spool = {j: p2.enter_context(
                tc.tile_pool(name=f"sp{j}", bufs=2, space="PSUM"))
                for j in range(MAC // 512)}
            cpool = p2.enter_context(tc.tile_pool(name="cp", bufs=2, space="PSUM"))
            wpool = {j: p2.enter_context(tc.tile_pool(name=f"wp{j}", bufs=4))
                     for j in range(MAC // 512)}
            upool = p2.enter_context(tc.tile_pool(name="up", bufs=3))
            rpool = p2.enter_context(tc.tile_pool(name="rp", bufs=2))
            opool = p2.enter_context(tc.tile_pool(name="op", bufs=3))

            cu_t = {}
            cun_t = {}
            rbc_t = {}
            pcu_t = {}

            def evac_slot(mac, slot):
                """Deferred mac-boundary work, one light piece per t-slot of
                the following mac: 0 -> psum->sbuf ctx_u copies; 1/2 -> per
                head: r = 1/sums row, rbc = ones^T r (PE broadcast down 64
                partitions), ctx_n = ctx_u * rbc. Normalizing ctx along the
                free (m) dim here removes both the per-chunk sums transposes
                and any per-partition scaling in the out-projection."""
                if slot == 0:
                    for h in range(HPC):
                        cu = upool.tile([HD + 1, MAC], dt.bfloat16, tag="cu",
                                        name=f"cu{h}")
                        nc.vector.tensor_copy(out=cu, in_=pcu_t[(mac, h)])
                        cu_t[(mac, h)] = cu
                elif slot in (1, 2):
                    h = slot - 1
                    cu = cu_t[(mac, h)]
                    rr = rpool.tile([128, MAC], dt.bfloat16, tag="rr",
                                    name="rr")
                    if mac == N_MAC - 1:
                        # tail: single-lane DVE reciprocal is ~6.5us; ACT is
                        # idle there, so use 1/s = exp(-ln(s)) (~2us, and
                        # sums ~4e3 are well-conditioned)
                        rl = rpool.tile([128, MAC], dt.float32, tag="rl",
                                        name="rl")
                        nc.scalar.activation(out=rl[HD:HD + 1, :],
                                             in_=cu[HD:HD + 1, :],
                                             func=AF.Ln)
                        nc.scalar.activation(out=rr[HD:HD + 1, :],
                                             in_=rl[HD:HD + 1, :],
                                             func=AF.Exp, scale=-1.0)
                    else:
                        with nc.allow_low_precision(
                                reason="softmax sums ~4e3; bf16 recip 0.4%"):
                            nc.vector.reciprocal(out=rr[HD:HD + 1, :],
                                                 in_=cu[HD:HD + 1, :])
                    rbc = cpool.tile([HD, MAC], dt.float32, tag="pc",
                                     name="rbc")
                    for j in range(MAC // 512):
                        js = slice(j * 512, (j + 1) * 512)
                        nc.tensor.matmul(rbc[:, js],
                                         lhsT=ones_bf[HD:HD + 1, :],
                                         rhs=rr[HD:HD + 1, js],
                                         start=True, stop=True)
                    if h == 0:
                        cun_t[mac] = upool.tile([128, MAC], dt.bfloat16,
                                                tag="cun", name="cun", bufs=4)
                        nc.vector.tensor_tensor(out=cun_t[mac][0:HD, :],
                                                in0=cu[0:HD, :],
                                                in1=rbc, op=ALU.mult)
                    else:
                        c1 = upool.tile([HD, MAC], dt.bfloat16, tag="c1",
                                        name="c1", bufs=2)
                        nc.vector.tensor_tensor(out=c1, in0=cu[0:HD, :],
                                                in1=rbc, op=ALU.mult)
                        cun_t[(mac, 1)] = c1
                elif slot == 3:
                    # PE-shift h1's normalized ctx to partitions [64:128) so
                    # the out-projection is one K=128 matmul per chunk
                    c1 = cun_t[(mac, 1)]
                    psh = cpool.tile([128, MAC], dt.float32, tag="pc",
                                     name="psh")
                    for j in range(MAC // 512):
                        js = slice(j * 512, (j + 1) * 512)
                        nc.tensor.matmul(psh[HD:128, js],
                                         lhsT=ident_bf[0:HD, 0:HD],
                                         rhs=c1[:, js],
                                         start=True, stop=True)
                    nc.vector.tensor_copy(out=cun_t[mac][HD:128, :],
                                          in_=psh[HD:128, :])

            for mac in range(N_MAC):
                m0 = mac * MAC
                for h in range(HPC):
                    pcu_t[(mac, h)] = cpool.tile([HD + 1, MAC], dt.float32,
                                                 tag="pc", name=f"pcu{h}")
                for t in range(TC):
                    ws = {}
                    for h in range(HPC):
                        hd0 = h * HD
                        for j in range(MAC // 512):
                            ps = spool[j].tile([128, 512], dt.float32,
                                               tag="s", name=f"ps{h}{j}")
                            nc.tensor.matmul(
                                ps,
                                lhsT=kf[hd0:hd0 + HD, t * 128:(t + 1) * 128],
                                rhs=qf[hd0:hd0 + HD,
                                       m0 + j * 512:m0 + (j + 1) * 512],
                                start=True, stop=True,
                                tile_position=(hd0, 0))
                            w = wpool[j].tile([128, 512], dt.float16,
                                              tag="w", name=f"w{h}{j}")
                            nc.scalar.activation(out=w, in_=ps,
                                                 func=AF.Exp, scale=SCALE)
                            ws[(h, j)] = w
                    for h in range(HPC):
                        for j in range(MAC // 512):
                            js = slice(j * 512, (j + 1) * 512)
                            nc.tensor.matmul(
                                pcu_t[(mac, h)][:, js],
                                lhsT=v_all[:, t, h, :],
                                rhs=ws[(h, j)],
                                start=(t == 0), stop=(t == TC - 1),
                                skip_group_check=True)
                    if mac > 0:
                        evac_slot(mac - 1, t)

            # ---------------- phase 3: out-projection tail ----------------
            # ctx is pre-normalized and both heads live on disjoint
            # partitions of one joint tile: out-proj is one K=128 matmul.
            for slot in range(4):
                evac_slot(N_MAC - 1, slot)
            for mac in range(N_MAC):
                for mc in range(MAC // 128):
                    ot = opool.tile([128, DIM], dt.bfloat16, tag="ot",
                                    name="ot")
                    for e in range(DIM // 512):
                        es = slice(e * 512, (e + 1) * 512)
                        ms = slice(mc * 128, (mc + 1) * 128)
                        po = spool[e].tile([128, 512], dt.float32, tag="s",
                                           name="po")
                        nc.tensor.matmul(po, lhsT=cun_t[mac][:, ms],
                                         rhs=woT[:, es],
                                         start=True, stop=True)
                        if e == 0:
                            nc.vector.tensor_copy(out=ot[:, es], in_=po)
                        else:
                            nc.scalar.copy(out=ot[:, es], in_=po)
                    r0 = mac * MAC + mc * 128
                    nc.sync.dma_start(out=out_d.ap()[r0:r0 + 128, :], in_=ot)

    nc.compile()
    return nc


def kernel(**inputs):
    global LAST_RESULT
    import ml_dtypes
    from concourse.bass_utils import run_bass_kernel_spmd

    x = np.asarray(inputs["x"], dtype=np.float32).reshape(S, DIM)
    ln_g = np.asarray(inputs["ln_g"], dtype=np.float32)
    ln_b = np.asarray(inputs["ln_b"], dtype=np.float32)
    Wq = np.asarray(inputs["Wq"], dtype=np.float32)
    Wk = np.asarray(inputs["Wk"], dtype=np.float32)
    Wv = np.asarray(inputs["Wv"], dtype=np.float32)
    Wo = np.asarray(inputs["Wo"], dtype=np.float32)
    bq = np.asarray(inputs["bq"], dtype=np.float32)
    bk = np.asarray(inputs["bk"], dtype=np.float32)
    bv = np.asarray(inputs["bv"], dtype=np.float32)
    bo = np.asarray(inputs["bo"], dtype=np.float32)

    if "nc" not in _CACHE:
        _CACHE["nc"] = _build()
    nc = _CACHE["nc"]

    bf16 = ml_dtypes.bfloat16
    in_maps = []
    for c in range(N_CORES):
        sl = slice(c * DC, (c + 1) * DC)
        in_maps.append({
            "x": x,
            "wqT": np.ascontiguousarray((Wq[sl] * ln_g[None, :]).T).astype(bf16),
            "wkT": np.ascontiguousarray((Wk[sl] * ln_g[None, :]).T).astype(bf16),
            "wvT": np.ascontiguousarray((Wv[sl] * ln_g[None, :]).T).astype(bf16),
            "woT": np.ascontiguousarray(Wo[:, sl].T).astype(bf16),
            "bq": bq[sl] + Wq[sl] @ ln_b,
            "bk": bk[sl] + Wk[sl] @ ln_b,
            "bv": bv[sl] + Wv[sl] @ ln_b,
        })

    res = run_bass_kernel_spmd(nc, in_maps, list(range(N_CORES)))
    LAST_RESULT = res

    acc = res.results[0]["out"].astype(np.float32)
    for c in range(1, N_CORES):
        acc = acc + res.results[c]["out"].astype(np.float32)
    acc += bo[None, :]
    return acc.reshape(B, S, DIM)
